# revision 34
# baseline (speedup 1.0000x reference)
"""MoE FFN (top-2 routing, 8 experts) on 8 Trainium2 NeuronCores.

Strategy (expert-pair x tensor-parallel hybrid):
  - Host computes router logits / top-2 / softmax (tiny: T x E) and
    gathers each expert's tokens.
  - Experts are paired (4 smallest loads with 4 largest) so each pair's
    token batch is ~T*K/4.  Each pair is served by TWO cores: both cores
    see all the pair's tokens, but each core holds only HALF of the F
    (FFN-intermediate) dimension of the pair's W1/W2.  F is the
    contraction dim of the second matmul, so each core emits a partial
    [H, C] output; the host sums the two partials (b2 is added on the
    even core only).  Per-core capacity is CA+CB ~ 2084 token-columns at
    half-F work == ~1042 token-equivalents, vs 1152 for plain expert
    parallelism: near-perfect balance across all 8 cores.
  - Core math (per chunk of <=512 token columns, fully transposed so
    weights are the stationary matmul operand):
        hT = GELU_tanh(W1s^T @ x + b1s)        [Fh, w]
        yT = wt * (W2s^T @ hT + b2s)           [H,  w]
    Matmul operands are fp16 (1 elem/cell/cycle on the PE, same as
    bf16); accumulation is fp32 in PSUM; bias/GELU/combine in fp32.
  - ALL DRAM tensors are pre-tiled on the host into [128, X] partition-
    major contiguous blocks, one block per DMA, so each dma_start is a
    trivial contiguous slice.  Rearranged (gathering) DMAs cost the
    issuing sequencer 0.7-5us of synchronous DIRECT2D descriptor
    generation EACH; with enough of them queued on the Activation
    engine's ring they delay the first GELU by ~25us and stall the PE.
  - DMA kicks are placed in strict need-order because the queues fair-
    share service among everything enqueued: the startup-critical
    xg0/w1A pieces lead the two HWDGE rings (SP + Activation, ~2 chains
    admitted per ring at a time), tiny combine-weight slices ride the
    GpSimd SWDGE, and the slot-B weights are interleaved into the
    y-output kick stream on the SP ring — output kicks block on compute
    data, so that bulk hits the queues only after ~45us.  (A kick
    placed behind an unblocked instruction stream issues immediately,
    so late kicks must ride behind genuinely-blocked instructions.)
  - The Activation engine issues only 3 early weight kicks, then pure
    GELU: its sequencer gates PSUM-A recycling, hence the PE pipeline.
  - ~28 dummy warmup matmuls lift the PE clock (HAM ramp: 1.2->2.4 GHz
    after ~3us of continuous busy) and bridge until the first x/W tiles
    land (~15us), so real matmuls start at full clock with no ramp-
    resetting idle gap.
  - Chunks within a capacity slot are equal-width (<=512) to avoid
    narrow-matmul instruction-overhead floors; b1==0/b2==0 fast paths
    skip the bias DMA and run the combine-multiply straight from PSUM
    on the DVE.
  - Host scatter-adds each pair's summed partials back into [T, H].
    Measured on trn2 (8 cores): ~150.7us HW exec (baseline expert-
    parallel fp16 implementation: ~168us); output max-abs error ~4e-4
    of output scale vs the fp32 reference.

Self-contained: hardcodes the problem shapes (H=768, F=3072, E=8, K=2).
"""

import os
import time

import numpy as np

H = 768
F = 3072
FH = F // 2          # per-core F slice
E = 8
K = 2
N_CORES = 8
P = 128
CHUNK = 512          # max token-chunk width (fp32 PSUM bank = 512 elems)

HK = H // P          # 6: contraction tiles for x@W1
HN = H // P          # 6: output row tiles of yT
FM = FH // P         # 12: F-half tiles (phase-A psum rows / phase-B k)
NW1 = 4              # DMA pieces per W1 slot-half
FQ = FH // NW1       # 384 cols per W1 piece
FQT = FQ // P        # 3 fm-tiles per piece
W2G = FM // 2        # 6 row-tiles per W2 group DMA
NCONST = 2 * FM + 2 * HN

PRECISION = os.environ.get("MOE_PRECISION", "fp16")  # "fp16" | "bf16" | "fp32"
N_WARM = int(os.environ.get("MOE_WARMUP", "23"))


def _chunk_plan(CA, CB):
    """Equal-width chunks (<=CHUNK) per capacity slot; returns
    [(col0, width, slot), ...]."""
    plan = []
    off = 0
    for s, C in ((0, CA), (1, CB)):
        k = max(1, -(-C // CHUNK))
        base, rem = divmod(C, k)
        for i in range(k):
            w = base + (1 if i < rem else 0)
            plan.append((off, w, s))
            off += w
    return plan


def _tile_pm(a):
    """[k*128, w] -> [128, k*w] partition-major contiguous block."""
    k = a.shape[0] // P
    return a.reshape(k, P, -1).transpose(1, 0, 2).reshape(P, -1)


# ---------------------------------------------------------------------------
# Bass/Tile device kernel
# ---------------------------------------------------------------------------

def _build_bass(CA, CB, zb1, zb2, precision=None):
    """Build + compile the per-core Bass program for slot capacities CA/CB.
    zb1/zb2: b1/b2 are identically zero (skip the bias inputs)."""
    from contextlib import ExitStack

    import concourse.bass as bass  # noqa: F401
    import concourse.tile as tile
    from concourse import bacc, mybir
    from concourse._compat import with_exitstack

    precision = precision or PRECISION
    CT = CA + CB
    f32 = mybir.dt.float32
    mdt = {"bf16": mybir.dt.bfloat16, "fp16": mybir.dt.float16,
           "fp32": f32}[precision]

    chunks = _chunk_plan(CA, CB)
    NCH = len(chunks)

    nc = bacc.Bacc("TRN2", target_bir_lowering=False, debug=False,
                   num_devices=N_CORES)
    # all DRAM tensors are [128, X]: partition-major tiled blocks
    xgt = nc.dram_tensor("xgt", [P, HK * CT], mdt, kind="ExternalInput").ap()
    # w1: 8 blocks (slot s, piece g): [P, HK*FQ] each
    w1 = nc.dram_tensor("w1", [P, 2 * NW1 * HK * FQ], mdt,
                        kind="ExternalInput").ap()
    # w2: 4 blocks (slot s, group g): [P, W2G*H] each
    w2 = nc.dram_tensor("w2", [P, 4 * W2G * H], mdt,
                        kind="ExternalInput").ap()
    # packed fp32 constants: [b1A(FM) | b1B(FM) | b2A(HN) | b2B(HN) | wt(CT)]
    cpk = nc.dram_tensor("cpk", [P, NCONST + CT], f32,
                         kind="ExternalInput").ap()
    # y: per (chunk, hn) blocks [P, w], chunk-major
    y = nc.dram_tensor("y", [P, HN * CT], f32, kind="ExternalOutput").ap()

    gelu = mybir.ActivationFunctionType.Gelu_apprx_tanh
    ident = mybir.ActivationFunctionType.Identity

    @with_exitstack
    def body(ctx: ExitStack, tc: tile.TileContext):
        const = ctx.enter_context(tc.tile_pool(name="const", bufs=1))
        w1p = ctx.enter_context(tc.tile_pool(name="w1p", bufs=1))
        w2p = ctx.enter_context(tc.tile_pool(name="w2p", bufs=1))
        xp = ctx.enter_context(tc.tile_pool(name="xp", bufs=1))
        hp = ctx.enter_context(tc.tile_pool(name="hp", bufs=1))
        yp = ctx.enter_context(tc.tile_pool(name="yp", bufs=3))
        psAp = ctx.enter_context(tc.tile_pool(name="psA", bufs=2, space="PSUM"))
        psBp = ctx.enter_context(tc.tile_pool(name="psB", bufs=6, space="PSUM"))

        # Pre-warm the PE's HAM clock gate during the DMA-bound startup:
        # dummy matmuls on a memset tile (no load dependency) lift the PE
        # clock 1.2 -> 2.4 GHz before the real data lands.
        wtile = xp.tile([P, CHUNK], mdt, tag="warm", name="warm")
        nc.vector.memset(wtile[:], 0.0)
        wps = psBp.tile([P, CHUNK], f32, tag="psB", name="warmps")
        for i in range(N_WARM):
            nc.tensor.matmul(wps[:], lhsT=wtile[:, 0:P], rhs=wtile[:],
                             start=(i == 0), stop=(i == N_WARM - 1))

        # ---------- input DMA kicks ----------
        xg = [None] * NCH
        xoff = [0] * NCH
        o = 0
        for i, (c0, w, _s) in enumerate(chunks):
            xoff[i] = o
            o += HK * w

        def load_xg(i, eng, eng2=None):
            c0, w, _s = chunks[i]
            t = xp.tile([P, HK * w], mdt, tag=f"xg{i}", name=f"xg{i}")
            if eng2 is None:
                eng.dma_start(t[:], xgt[:, xoff[i]:xoff[i] + HK * w])
            else:
                # split across two rings to halve the critical latency
                half = (HK // 2) * w
                eng.dma_start(t[:, :half], xgt[:, xoff[i]:xoff[i] + half])
                eng2.dma_start(t[:, half:],
                               xgt[:, xoff[i] + half:xoff[i] + HK * w])
            xg[i] = t

        w1q = [None] * (2 * NW1)

        def load_w1(s, g, eng):
            tq = w1p.tile([P, HK * FQ], mdt, tag=f"w1q{s}_{g}",
                          name=f"w1q{s}_{g}")
            o1 = (s * NW1 + g) * HK * FQ
            eng.dma_start(tq[:], w1[:, o1:o1 + HK * FQ])
            w1q[s * NW1 + g] = tq

        w2g = [None] * 4

        def load_w2(s, g, eng):
            tg = w2p.tile([P, W2G * H], mdt, tag=f"w2g{s}_{g}",
                          name=f"w2g{s}_{g}")
            o2 = (s * 2 + g) * W2G * H
            eng.dma_start(tg[:], w2[:, o2:o2 + W2G * H])
            w2g[s * 2 + g] = tg

        wtc = [None] * NCH

        def load_wt(i, eng):
            c0, w, _s = chunks[i]
            t = const.tile([P, w], f32, name=f"wtc{i}")
            eng.dma_start(t[:], cpk[:, NCONST + c0:NCONST + c0 + w])
            wtc[i] = t

        # The 16 DMA queues process descriptors roughly FIFO, so enqueue
        # strictly in need-order and keep bulk OFF the queues early:
        #   sync:   xg0, w1A2, xg1, w2A0, xg2.. (need-ordered);
        #   scalar: w1A0 (parallel with xg0 -> first matmul ~5us sooner),
        #           w1A1, w1A3, w2A1 — 4 cheap kicks, then pure GELU;
        #   gpsimd: tiny wt/bias only;
        #   slot-B weights: kicks interleaved into the y-output stream in
        #           phase B below — output kicks block on compute data,
        #           so these bulk loads hit the queues only after ~45us,
        #           well clear of the startup-critical window.  (Kicks
        #           must ride an engine whose NEXT instruction is itself
        #           blocked — a kick placed behind an unblocked stream
        #           gets hoisted to issue immediately.)
        # Each HWDGE ring admits only ~2 descriptor chains at a time
        # (later kicks block on ring credit), so the piece each phase-A
        # group needs next must be at the FRONT of some ring: w1A1 rides
        # sync directly behind xg0 (admitted together), w1A0/w1A3 lead
        # the scalar ring.
        load_xg(0, nc.sync)
        load_w1(0, 0, nc.scalar)
        bias = None
        if not (zb1 and zb2):
            bias = const.tile([P, NCONST], f32, name="bias")
            nc.gpsimd.dma_start(bias[:], cpk[:, 0:NCONST])
        # tiny wt slices as the SECOND admitted chain on each HWDGE ring:
        # rings admit ~2 chains at once and the queues fair-share, so a
        # small second chain leaves nearly all early bandwidth to the
        # startup-critical xg0 + w1A piece0.
        load_wt(0, nc.sync)
        if NCH > 1:
            load_wt(1, nc.scalar)
        load_w1(0, 1, nc.sync)
        load_w1(0, 3, nc.scalar)
        load_w1(0, 2, nc.sync)
        if NCH > 1:
            load_xg(1, nc.sync)
        load_w2(0, 1, nc.scalar)
        load_w2(0, 0, nc.sync)
        for i in range(2, NCH):
            load_xg(i, nc.sync)
        for i in range(2, NCH):
            load_wt(i, nc.gpsimd)
        mid_kicks = []
        if NCH >= 2:
            # slot-B weight kicks, deferred into the output stream (sync
            # ring); all six fit in phase-B(0)'s six output slots, so
            # they are registered before any slot-B phase is emitted.
            late_kicks = (
                [lambda g=g: load_w1(1, g, nc.sync) for g in range(NW1)]
                + [lambda g=g: load_w2(1, g, nc.sync) for g in range(2)]
            )
        else:
            for g in range(NW1):
                load_w1(1, g, nc.sync)
            for g in range(2):
                load_w2(1, g, nc.sync)
            late_kicks = []

        def b1v(s, fm):
            return bias[:, s * FM + fm:s * FM + fm + 1]

        def b2v(s, hn):
            return bias[:, 2 * FM + s * HN + hn:2 * FM + s * HN + hn + 1]

        def w1_tile(s, hk, fm):
            q = w1q[s * NW1 + fm // FQT]
            o1 = hk * FQ + (fm % FQT) * P
            return q[:, o1:o1 + P]

        def w2_tile(s, fk, hn):
            g = w2g[s * 2 + fk // W2G]
            o2 = (fk % W2G) * H + hn * P
            return g[:, o2:o2 + P]

        yoffs = [0] * NCH
        o = 0
        for i, (c0, w, _s) in enumerate(chunks):
            yoffs[i] = o
            o += HN * w

        h_all = [None] * NCH

        def phase_a(ci):
            # ---- phase A: hT[f, c] = gelu((x@W1s)[c, f] + b1s[f]) ----
            c0, w, s = chunks[ci]
            hts = [None] * FM
            for fm in range(FM):
                ps = psAp.tile([P, CHUNK], f32, tag="psA", name="psA")
                for hk in range(HK):
                    nc.tensor.matmul(
                        ps[:, :w],
                        lhsT=w1_tile(s, hk, fm),
                        rhs=xg[ci][:, hk * w:(hk + 1) * w],
                        start=(hk == 0), stop=(hk == HK - 1),
                    )
                ht = hp.tile([P, CHUNK], mdt, tag=f"hts{fm}_{ci % 2}",
                             name=f"hts{fm}_{ci % 2}")
                if zb1:
                    nc.scalar.activation(ht[:, :w], ps[:, :w], gelu)
                else:
                    nc.scalar.activation(ht[:, :w], ps[:, :w], gelu,
                                         bias=b1v(s, fm))
                hts[fm] = ht
                if ci == 0 and fm % 2 == 1 and mid_kicks:
                    mid_kicks.pop(0)()
            h_all[ci] = hts

        def phase_b(ci):
            # ---- phase B: yT[h, c] = sum_f W2s[f, h] * hT[f, c] ----
            c0, w, s = chunks[ci]
            hts = h_all[ci]
            for hn in range(HN):
                ps = psBp.tile([P, CHUNK], f32, tag="psB", name="psB")
                for fk in range(FM):
                    nc.tensor.matmul(
                        ps[:, :w],
                        lhsT=w2_tile(s, fk, hn),
                        rhs=hts[fk][:, :w],
                        start=(fk == 0), stop=(fk == FM - 1),
                    )
                # ---- epilogue: (+b2), (*wt), store ----
                if zb2:
                    # b2 == 0: multiply straight out of PSUM on the DVE,
                    # keeping the Activation engine free for GELUs.
                    ot2 = yp.tile([P, CHUNK], f32, tag="yout2",
                                  name="yout2")
                    nc.vector.tensor_mul(ot2[:, :w], ps[:, :w], wtc[ci][:])
                else:
                    ot = yp.tile([P, CHUNK], f32, tag="yout", name="yout")
                    nc.scalar.activation(ot[:, :w], ps[:, :w], ident,
                                         bias=b2v(s, hn))
                    ot2 = yp.tile([P, CHUNK], f32, tag="yout2",
                                  name="yout2")
                    nc.vector.tensor_mul(ot2[:, :w], ot[:, :w], wtc[ci][:])
                nc.sync.dma_start(y[:, yoffs[ci] + hn * w:
                                    yoffs[ci] + (hn + 1) * w], ot2[:, :w])
                if late_kicks:
                    late_kicks.pop(0)()

        # Software-pipelined schedule with one-chunk phase-A lookahead
        # (A0 A1 B0 A2 B1 ... B_last): phase B of chunk c issues on the
        # PE only after phase A of chunk c+1, so B never chases a weight
        # or h-activation dependency.
        if NCH >= 2:
            phase_a(0)
            phase_a(1)
            for ci in range(NCH):
                phase_b(ci)
                if ci + 2 < NCH:
                    phase_a(ci + 2)
        else:
            phase_a(0)
            phase_b(0)

    with tile.TileContext(nc) as tc:
        body(tc)
    nc.compile()
    return nc


# ---------------------------------------------------------------------------
# Host-side routing + dispatch
# ---------------------------------------------------------------------------

def _route(xf, gate_w):
    """Top-2 router in float64 for a numerically robust top-k set.

    Returns per-expert (token_idx, weight) lists.
    """
    logits = xf.astype(np.float64) @ gate_w.astype(np.float64)  # [T, E]
    top_idx = np.argpartition(logits, E - K, axis=1)[:, E - K:]  # [T, K]
    top_val = np.take_along_axis(logits, top_idx, axis=1)
    m = top_val.max(axis=1, keepdims=True)
    ex = np.exp(top_val - m)
    wts = ex / ex.sum(axis=1, keepdims=True)  # [T, K] float64

    toks, ws = [], []
    for e in range(E):
        mask = top_idx == e  # [T, K]
        rows = np.nonzero(mask.any(axis=1))[0]
        toks.append(rows)
        ws.append(wts[mask].astype(np.float32))
    return toks, ws


def _np_mdt():
    import ml_dtypes
    return {"bf16": ml_dtypes.bfloat16, "fp16": np.float16,
            "fp32": np.float32}[PRECISION]


def _make_in_maps(xf, gate_w, W1, b1, W2, b2):
    toks, ws = _route(xf, gate_w)
    n = [len(t) for t in toks]
    order = list(np.argsort(n))
    pairs = [(order[i], order[E - 1 - i]) for i in range(E // 2)]
    CA = max(1, max(n[a] for a, _ in pairs))
    CB = max(1, max(n[b] for _, b in pairs))
    CT = CA + CB
    chunks = _chunk_plan(CA, CB)
    mdt = _np_mdt()

    W1a = np.asarray(W1, np.float32)
    b1a = np.asarray(b1, np.float32)
    W2a = np.asarray(W2, np.float32)
    b2a = np.asarray(b2, np.float32)
    in_maps = []
    for p_i, (a, b) in enumerate(pairs):
        xgT = np.zeros((H, CT), mdt)
        xgT[:, :n[a]] = xf[toks[a]].T.astype(mdt)
        xgT[:, CA:CA + n[b]] = xf[toks[b]].T.astype(mdt)
        # chunk-major partition-tiled x blocks
        xgt = np.concatenate(
            [_tile_pm(xgT[:, c0:c0 + w]) for c0, w, _s in chunks], axis=1)
        wtb = np.zeros((P, CT), np.float32)
        wtb[:, :n[a]] = ws[a][None, :]
        wtb[:, CA:CA + n[b]] = ws[b][None, :]
        for half in range(2):
            fc = slice(half * FH, (half + 1) * FH)
            w1blk, w2blk = [], []
            for e in (a, b):
                w1h = W1a[e][:, fc].astype(mdt)          # [H, FH]
                for g in range(NW1):
                    w1blk.append(_tile_pm(w1h[:, g * FQ:(g + 1) * FQ]))
                w2h = W2a[e][fc, :].astype(mdt)          # [FH, H]
                for g in range(2):
                    w2blk.append(_tile_pm(
                        w2h[g * W2G * P:(g + 1) * W2G * P, :]))
            w1pk = np.concatenate(w1blk, axis=1)
            w2pk = np.concatenate(w2blk, axis=1)
            # b2 only on the even core (partials are summed on host)
            b2c = (b2a if half == 0 else np.zeros_like(b2a))
            cpk = np.concatenate([
                b1a[a][fc].reshape(FM, P).T,
                b1a[b][fc].reshape(FM, P).T,
                b2c[a].reshape(HN, P).T,
                b2c[b].reshape(HN, P).T,
                wtb,
            ], axis=1)
            in_maps.append({
                "xgt": np.ascontiguousarray(xgt),
                "w1": np.ascontiguousarray(w1pk),
                "w2": np.ascontiguousarray(w2pk),
                "cpk": np.ascontiguousarray(cpk),
            })
    return in_maps, toks, pairs, n, CA, CB


def _untile_y(ydram, chunks, CT):
    """[P, HN*CT] chunk-major blocks -> [H, CT]."""
    yf = np.empty((H, CT), np.float32)
    o = 0
    for c0, w, _s in chunks:
        for hn in range(HN):
            yf[hn * P:(hn + 1) * P, c0:c0 + w] = ydram[:, o:o + w]
            o += w
    return yf


def _run(inputs, trace=False):
    global PRECISION
    from concourse.bass_utils import run_bass_kernel_spmd

    x, gate_w, W1, b1, W2, b2 = (inputs[k] for k in
                                 ("x", "gate_w", "W1", "b1", "W2", "b2"))
    x = np.asarray(x)
    Bb, S, Hd = x.shape
    assert Hd == H
    T = Bb * S
    xf = np.ascontiguousarray(x.reshape(T, Hd), dtype=np.float32)
    gate_w = np.asarray(gate_w, np.float32)

    # fp16 matmul operands need moderate dynamic range; fall back to
    # bf16 (full fp32 exponent range) if the data is far outside the
    # expected unit-scale regime.
    if PRECISION == "fp16":
        amax = max(float(np.abs(np.asarray(t)).max())
                   for t in (xf, W1, W2))
        if not np.isfinite(amax) or amax > 1e3:
            PRECISION = "bf16"

    in_maps, toks, pairs, n, CA, CB = _make_in_maps(
        xf, gate_w, W1, b1, W2, b2)
    CT = CA + CB
    chunks = _chunk_plan(CA, CB)
    zb1 = not np.any(np.asarray(b1))
    zb2 = not np.any(np.asarray(b2))
    nc = _build_bass(CA, CB, zb1, zb2)

    kwargs = {}
    if trace:
        kwargs = dict(trace=True, trace_cores=list(range(N_CORES)))
    try:
        res = run_bass_kernel_spmd(nc, in_maps, core_ids=list(range(N_CORES)),
                                   **kwargs)
    except Exception:
        # One retry for transient device faults.
        time.sleep(5)
        res = run_bass_kernel_spmd(nc, in_maps, core_ids=list(range(N_CORES)),
                                   **kwargs)
    out = np.zeros((T, H), np.float32)
    for p_i, (a, b) in enumerate(pairs):
        y0 = _untile_y(res.results[2 * p_i]["y"], chunks, CT)
        y1 = _untile_y(res.results[2 * p_i + 1]["y"], chunks, CT)
        ysum = y0 + y1
        out[toks[a]] += ysum[:, :n[a]].T
        out[toks[b]] += ysum[:, CA:CA + n[b]].T
    return out.reshape(Bb, S, Hd), res


def kernel(x, gate_w, W1, b1, W2, b2):
    out, _ = _run({"x": x, "gate_w": gate_w, "W1": W1, "b1": b1,
                   "W2": W2, "b2": b2})
    return out.astype(np.asarray(x).dtype, copy=False)


# Exposed for test.py: run with profiling, return (output, BassKernelResults)
def kernel_profiled(x, gate_w, W1, b1, W2, b2):
    return _run({"x": x, "gate_w": gate_w, "W1": W1, "b1": b1,
                 "W2": W2, "b2": b2}, trace=True)
